# revision 33
# baseline (speedup 1.0000x reference)
"""Trainium2 Bass kernel for nn_FC_STGNN_SSC (STGNN over conv sleep-features).

Data-parallel over the batch: each of 8 NeuronCores processes 4 of the 32
batch elements (96 of the 768 flattened conv rows). All weights replicated.

Per-core pipeline (conv matmuls fp32r; map2/fc1 weights bf16):
  conv1(1->64,k3)  -> BN+ReLU -> maxpool   2-row block-diag matmuls (K=6, M=128)
  conv2(64->128)   -> BN+ReLU -> maxpool   3 tap-passes K=64 (dup'd weight halves)
  conv3(128->128)  -> BN+ReLU -> maxpool   3 tap-passes K=128
  map2 24448->256: l-major contraction; W' streamed bf16 through an SBUF ring
  +BN+posenc; 2 MPNN blocks (graph logits via free-sliced Gram matmuls,
  message passing as one K=96 matmul against a scattered ADJBIG);
  4-layer FC head fully transposed (h^T = W-chunk^T @ prev^T, no transposes).

The conv loop is software-pipelined by emission order: conv1 runs two pairs
ahead of conv2/conv3 on PE so the act->pool chain (Act full-width BN+ReLU,
DVE maxpool; ~1/3 of conv3-even rows take a DVE BN path to keep Act < PE)
never stalls PE. Engine legality (walrus, not CoreSim): TensorTensor reads
at most one PSUM input; gpsimd is DMA/memset-only; DMA queues exist on
SP/gpsimd/Activation only. PSUM is exactly 8 banks: ps1 3 + ps2 2x2 + ps3 1.
wprime prefetches 14/24 tiles during conv (SP), streams the rest during
map2 on SP+gpsimd; fc1 weights prefetch before the MPNN blocks.
"""

import os
import sys

import numpy as np

if not any(os.path.isdir(os.path.join(p, "concourse")) for p in sys.path if p):
    sys.path.insert(0, "/opt/trn_rl_repo")

import concourse.bass as bass  # noqa: E402,F401
import concourse.bacc as bacc  # noqa: E402
import concourse.mybir as mybir  # noqa: E402
import concourse.tile as tile  # noqa: E402

F32 = mybir.dt.float32
F32R = mybir.dt.float32r
BF16 = mybir.dt.bfloat16
ACTF = mybir.ActivationFunctionType
AL = mybir.AluOpType

# model dims
BS, TLEN, NNODE, DIM = 32, 6, 4, 1500
HID = 128
D2 = 256
LSTMH, LSTMO, KCONV = 64, 128, 3
CONV_OUT = 191
DECAY = 0.7
FEAT_IN = LSTMO * CONV_OUT  # 24448

NCORES = 8
BSH = BS // NCORES          # 4 batch elems per core
R = BSH * TLEN * NNODE      # 96 conv rows per core
PAIRS = R // 2              # 48
PGRP = 1                    # conv1 pairs per T1 tile

L1, P1 = 1500, 751
L2, P2 = 753, 377
L3, P3 = 381, 191

# const-vector column indices within the cv section
(C_S1, C_B1, C_S2, C_B2, C_S3, C_B3,
 C_SA1_0, C_SA1_1, C_BA1_0, C_BA1_1,
 C_SA2_0, C_SA2_1, C_BA2_0, C_BA2_1,
 C_SM1, C_BM1, C_SM2, C_BM2,
 C_FB1_0, C_FB1_1, C_FB2_0, C_FB2_1, C_FB3, NCV) = range(24)

GRAPHS1 = [(b, j) for b in range(BSH) for j in range(5)]   # stride 1, nw=5
GRAPHS2 = [(b, j) for b in range(BSH) for j in range(3)]   # stride 2, nw=3
NG1, NG2 = len(GRAPHS1), len(GRAPHS2)                       # 20, 12


# ---- const blob layouts: name -> (col offset, ncols)
def _mk_layout(sections):
    off, lay = 0, {}
    for name, w in sections:
        lay[name] = (off, w)
        off += w
    return lay, off


BR_LAY, BR_COLS = _mk_layout([
    ("b1blk", 128), ("w2t", 384), ("w3t", 384),
    ("g1w", 512), ("g2w", 512), ("th1w", 256), ("th2w", 256),
    ("fc2w", 512), ("fc3w", 256), ("fc4w", 8),
])
BF_LAY, BF_COLS = _mk_layout([
    ("cv", NCV), ("ident", 128), ("mask8", 8 * NG1), ("eyeneg", 8 * NG1),
    ("eyepos", 8 * NG1), ("fb4", 8), ("sm2rep", D2), ("btot", D2),
    ("g1brep", D2), ("g2brep", D2),
])

W_TILE = 8          # wprime chunks per DMA
N_WTILES = (P3 + W_TILE - 1) // W_TILE   # 24


def r32(x):
    return np.ascontiguousarray(x, dtype=np.float32)


def round_fp32r(x):
    """fp32 -> fp32r (11-bit mantissa, RNE); matches walrus fp32_to_fp32r."""
    u = np.ascontiguousarray(x, np.float32).view(np.uint32).astype(np.uint64)
    r = ((u + 0x7FF + ((u >> 12) & 1)) & 0xFFFFF000).astype(np.uint32)
    return r.view(np.float32).reshape(np.shape(x))


def fold_bn(p, extra_bias=None, post_scale=1.0):
    """y = x*scale + bias  ==  post_scale * BN(x + extra_bias)."""
    g, be, m, v = (np.asarray(p[i], np.float64) for i in range(4))
    sc = g / np.sqrt(v + 1e-5)
    bi = be - m * sc
    if extra_bias is not None:
        bi = bi + np.asarray(extra_bias, np.float64) * sc
    return r32(sc * post_scale), r32(bi * post_scale)


def pos_encoding():
    pos = np.arange(TLEN, dtype=np.float32)[:, None]
    div = np.exp(np.arange(0, D2, 2, dtype=np.float32)
                 * (np.float32(-np.log(np.float32(100.0))) / np.float32(D2)))
    pe = np.zeros((TLEN, D2), np.float32)
    pe[:, 0::2] = np.sin(pos * div)
    pe[:, 1::2] = np.cos(pos * div)
    return pe


def prep_consts(inp):
    """Host-side constant prep (shared by all cores)."""
    br = np.zeros((128, BR_COLS), np.float32)

    def brput(name, rows, arr):
        c0, w = BR_LAY[name]
        a = np.asarray(arr, np.float32)
        assert a.shape == (rows, w), (name, a.shape)
        br[0:rows, c0:c0 + w] = a

    w2 = np.asarray(inp["conv2_w"], np.float32)
    w2t = np.zeros((128, 384), np.float32)
    for t in range(KCONV):
        blk = w2[:, :, t].T
        w2t[0:64, 128 * t:128 * (t + 1)] = blk
        w2t[64:128, 128 * t:128 * (t + 1)] = blk
    brput("w2t", 128, w2t)

    w3 = np.asarray(inp["conv3_w"], np.float32)
    w3t = np.zeros((128, 384), np.float32)
    for t in range(KCONV):
        w3t[:, 128 * t:128 * (t + 1)] = w3[:, :, t].T
    brput("w3t", 128, w3t)

    w1 = np.asarray(inp["conv1_w"], np.float32)
    b1blk = np.zeros((128, 128), np.float32)
    for ri in range(2):
        for t in range(KCONV):
            b1blk[3 * ri + t, 64 * ri:64 * ri + 64] = w1[:, 0, t]
    brput("b1blk", 128, b1blk)

    def chunks2(a):     # [256, W] -> [128, 2W]
        a = np.asarray(a, np.float32)
        return np.concatenate([a[0:128], a[128:256]], axis=1)

    brput("g1w", 128, chunks2(inp["g1_w"]))
    brput("g2w", 128, chunks2(inp["g2_w"]))
    brput("th1w", 128, chunks2(inp["th1_w"]))
    brput("th2w", 128, chunks2(inp["th2_w"]))
    brput("fc2w", 128, chunks2(inp["fc2_w"]))
    brput("fc3w", 128, chunks2(inp["fc3_w"]))
    brput("fc4w", 128, np.pad(np.asarray(inp["fc4_w"], np.float32),
                              ((0, 0), (0, 3))))
    br = round_fp32r(br)

    bf = np.zeros((128, BF_COLS), np.float32)

    def bfput(name, rows, arr):
        c0, w = BF_LAY[name]
        a = np.asarray(arr, np.float32)
        assert a.shape == (rows, w), (name, a.shape)
        bf[0:rows, c0:c0 + w] = a

    s1, b1 = fold_bn(inp["bn1"])
    s2, b2 = fold_bn(inp["bn2"])
    s3, b3 = fold_bn(inp["bn3"])
    assert (s1 > 0).all() and (s2 > 0).all() and (s3 > 0).all()
    sa1, ba1 = fold_bn(inp["bnA1"])
    sa2, ba2 = fold_bn(inp["bnA2"])
    sm1f, bm1f = fold_bn(inp["bnM1"], extra_bias=inp["th1_b"], post_scale=0.5)
    sm2f, bm2f = fold_bn(inp["bnM2"], extra_bias=inp["th2_b"], post_scale=0.5)

    cv = np.zeros((128, NCV), np.float32)
    cv[:, C_S1] = np.concatenate([s1, s1])
    cv[:, C_B1] = np.concatenate([b1, b1])
    cv[:, C_S2], cv[:, C_B2] = s2, b2
    cv[:, C_S3], cv[:, C_B3] = s3, b3
    cv[:, C_SA1_0], cv[:, C_SA1_1] = sa1[0:128], sa1[128:256]
    cv[:, C_BA1_0], cv[:, C_BA1_1] = ba1[0:128], ba1[128:256]
    cv[:, C_SA2_0], cv[:, C_SA2_1] = sa2[0:128], sa2[128:256]
    cv[:, C_BA2_0], cv[:, C_BA2_1] = ba2[0:128], ba2[128:256]
    cv[:, C_SM1], cv[:, C_BM1] = sm1f, bm1f
    cv[:, C_SM2], cv[:, C_BM2] = sm2f, bm2f
    fb1 = np.asarray(inp["fc1_b"], np.float32)
    fb2 = np.asarray(inp["fc2_b"], np.float32)
    cv[:, C_FB1_0], cv[:, C_FB1_1] = fb1[0:128], fb1[128:256]
    cv[:, C_FB2_0], cv[:, C_FB2_1] = fb2[0:128], fb2[128:256]
    cv[:, C_FB3] = np.asarray(inp["fc3_b"], np.float32)
    bfput("cv", 128, cv)
    bfput("ident", 128, np.eye(128, dtype=np.float32))

    t_of = np.arange(8) // NNODE
    mask8 = DECAY ** np.abs(t_of[:, None] - t_of[None, :]).astype(np.float32)
    eye8 = np.eye(8, dtype=np.float32)
    bfput("mask8", 8, np.tile(mask8, (1, NG1)))
    bfput("eyeneg", 8, np.tile(eye8 * np.float32(-1e8), (1, NG1)))
    bfput("eyepos", 8, np.tile(eye8, (1, NG1)))
    bfput("fb4", BSH, np.tile(np.pad(np.asarray(inp["fc4_b"], np.float32),
                                     (0, 3))[None, :], (BSH, 1)))

    g, be, m, v = (np.asarray(inp["bn_map2"][i], np.float64) for i in range(4))
    sc = g / np.sqrt(v + 1e-5)
    bi = be + (np.asarray(inp["map2_b"], np.float64) - m) * sc
    bfput("sm2rep", R, np.tile(r32(sc)[None, :], (R, 1)))
    pe = pos_encoding().astype(np.float64)
    btot = np.zeros((R, D2), np.float64)
    for r in range(R):
        btot[r] = bi + pe[(r // NNODE) % TLEN]
    bfput("btot", R, r32(btot))
    bfput("g1brep", R, np.tile(np.asarray(inp["g1_b"], np.float32)[None, :], (R, 1)))
    bfput("g2brep", R, np.tile(np.asarray(inp["g2_b"], np.float32)[None, :], (R, 1)))

    import ml_dtypes
    mw = np.asarray(inp["map2_w"], np.float32).reshape(LSTMO, CONV_OUT, D2)
    wprime = mw.transpose(1, 0, 2).reshape(FEAT_IN, D2).astype(ml_dtypes.bfloat16)

    return {
        "blob_r": br, "blob_f": bf, "wprime": wprime,
        "fc1w": np.asarray(inp["fc1_w"], np.float32).astype(ml_dtypes.bfloat16),
    }


def _mm(nc, out, lhsT, rhs, **kw):
    def cast(x):
        return x if x.dtype == BF16 else x.bitcast(F32R)
    nc.tensor.matmul(out, cast(lhsT), cast(rhs), **kw)


def build_program(n_wring=14):
    from contextlib import ExitStack

    nc = bacc.Bacc("TRN2", target_bir_lowering=False, debug=False)

    t1all_d = nc.dram_tensor("t1all", [3 * R, 1502], F32R, kind="ExternalInput")
    br_d = nc.dram_tensor("blob_r", [128, BR_COLS], F32R, kind="ExternalInput")
    bf_d = nc.dram_tensor("blob_f", [128, BF_COLS], F32, kind="ExternalInput")
    wp_d = nc.dram_tensor("wprime", [FEAT_IN, D2], BF16, kind="ExternalInput")
    f1_d = nc.dram_tensor("fc1w", [4096, D2], BF16, kind="ExternalInput")
    out_d = nc.dram_tensor("out", [BSH, 5], F32, kind="ExternalOutput")

    with tile.TileContext(nc) as tc, ExitStack() as st:
        persist = st.enter_context(tc.tile_pool(name="persist", bufs=1))

        brsb = persist.tile([128, BR_COLS], F32R)
        bfsb = persist.tile([128, BF_COLS], F32)
        # small heads first so conv1 (b1blk) and act1 (cv) start early;
        # the big tails are emitted after the first t1 loads (see below)
        nc.sync.dma_start(brsb[:, 0:128], br_d[:, 0:128])
        nc.sync.dma_start(bfsb[:, 0:NCV], bf_d[:, 0:NCV])

        def load_blob_tails():
            nc.sync.dma_start(brsb[:, 128:BR_COLS], br_d[:, 128:BR_COLS])
            nc.sync.dma_start(bfsb[:, NCV:BF_COLS], bf_d[:, NCV:BF_COLS])

        def brs(name, rows=128):            # fp32r blob slice
            c0, w = BR_LAY[name]
            return brsb[0:rows, c0:c0 + w]

        def bfs(name, rows=128):            # fp32 blob slice
            c0, w = BF_LAY[name]
            return bfsb[0:rows, c0:c0 + w]

        def cvcol(i):
            c0, _ = BF_LAY["cv"]
            return bfsb[:, c0 + i:c0 + i + 1]

        identsb = bfs("ident")

        a3 = persist.tile([128, R * P3], BF16)          # conv3 out, [o, r*191+l]
        a3v = a3.rearrange("p (r l) -> p r l", l=P3)

        # ======== conv stack ========
        wring_ctx = tc.tile_pool(name="wring", bufs=15)
        wring = st.enter_context(wring_ctx)
        wtiles = {}

        def wp_load(wi):
            l0 = W_TILE * wi
            nch = min(W_TILE, P3 - l0)
            wt = wring.tile([128, W_TILE * D2], BF16, tag="wp")
            # first 14 loads happen during conv where SP is idle; the
            # map2-time stream alternates so neither queue serializes
            eng = nc.sync if (wi < 14 or wi % 2 == 0) else nc.gpsimd
            eng.dma_start(
                wt.rearrange("p (n c) -> p n c", c=D2)[:, 0:nch, :],
                wp_d[128 * l0:128 * (l0 + nch), :]
                .rearrange("(n p) c -> p n c", p=128),
            )
            wtiles[wi] = wt

        with (
            tc.tile_pool(name="t1", bufs=4) as t1p,
            tc.tile_pool(name="rw1", bufs=2) as rw1p,
            tc.tile_pool(name="p1", bufs=3) as p1p,
            tc.tile_pool(name="rw2", bufs=2) as rw2p,
            tc.tile_pool(name="s2", bufs=3) as s2p,
            tc.tile_pool(name="rw3", bufs=2) as rw3p,
            tc.tile_pool(name="psc1", bufs=1, space="PSUM") as psc1,
            tc.tile_pool(name="psc2", bufs=2, space="PSUM") as psc2,
            tc.tile_pool(name="psc3", bufs=1, space="PSUM") as psc3,
        ):
            t1s, ps1s, p1s = {}, {}, {}

            def t1_load(p):
                t1 = t1p.tile([6, 1502], F32R, tag="t1")
                nc.gpsimd.dma_start(
                    t1.rearrange("k (pl c) -> k pl c", c=1502)[:],
                    t1all_d[6 * p:6 * (p + 1), :]
                    .rearrange("(pl k) c -> k pl c", k=6),
                )
                t1s[p] = t1

            def conv1(p):
                t1 = t1s.pop(p)
                ps1 = psc1.tile([128, L1], F32)
                for c0, c1 in ((0, 512), (512, 1024), (1024, L1)):
                    _mm(nc, ps1[:, c0:c1], brs("b1blk", 6), t1[:, c0:c1])
                ps1s[p] = ps1

            def pa1(p):
                # maxpool on raw conv1 PSUM (BN scale>0 so pool commutes),
                # then one fused BN+ReLU activation on the pooled 751 cols
                ps1 = ps1s.pop(p)
                e1 = rw1p.tile([128, L1], F32, tag="e1")
                nc.scalar.activation(e1[:], ps1[:], ACTF.Relu,
                                     bias=cvcol(C_B1), scale=cvcol(C_S1))
                p1t = p1p.tile([128, 771], F32R, tag="p1t")
                nc.gpsimd.memset(p1t[:, 0:2].bitcast(F32), 0.0)
                nc.gpsimd.memset(p1t[:, 753:771].bitcast(F32), 0.0)
                nc.vector.tensor_max(p1t[:, 3:752],
                                     e1[:, 1:1498:2], e1[:, 2:1499:2])
                nc.vector.tensor_copy(p1t[:, 2:753:750], e1[:, 0:L1:L1 - 1])
                p1s[p] = p1t

            def conv2(p1t, ri):
                base = 64 * ri
                ps2 = psc2.tile([128, 768], F32)
                for t in range(KCONV):
                    lhs = brs("w2t")[base:base + 64, 128 * t:128 * (t + 1)]
                    _mm(nc, ps2[:, 0:512], lhs, p1t[base:base + 64, t:t + 512],
                        start=(t == 0), stop=(t == 2))
                    _mm(nc, ps2[:, 512:768], lhs,
                        p1t[base:base + 64, t + 512:t + 768],
                        start=(t == 0), stop=(t == 2))
                return ps2

            def pa2(ps2, ri):
                # BN on Act for row0, on DVE for row1 (keeps Act under the
                # PE per-pair budget); maxpool on gpsimd (SBUF-only there)
                s2t = s2p.tile([128, L3 + 3], F32R, tag="s2t")
                nc.gpsimd.memset(s2t[:, 0:3].bitcast(F32), 0.0)
                nc.gpsimd.memset(s2t[:, 380:384].bitcast(F32), 0.0)
                e2 = rw2p.tile([128, L2], F32, tag=f"e2{ri}")
                nc.scalar.activation(e2[:], ps2[:, 0:L2], ACTF.Relu,
                                     bias=cvcol(C_B2), scale=cvcol(C_S2))
                nc.vector.tensor_max(s2t[:, 4:380],
                                     e2[:, 1:752:2], e2[:, 2:753:2])
                nc.vector.tensor_copy(s2t[:, 3:4], e2[:, 0:1])
                return s2t

            def conv3(s2t):
                ps3 = psc3.tile([128, L3 + 1], F32)
                for t in range(KCONV):
                    _mm(nc, ps3[:], brs("w3t")[:, 128 * t:128 * (t + 1)],
                        s2t[:, t:t + L3 + 1], start=(t == 0), stop=(t == 2))
                return ps3

            def pa3(ps3, r):
                e3 = rw3p.tile([128, L3], F32, tag=f"e3{r % 2}")
                dst1 = a3[:, r * P3 + 1:(r + 1) * P3]
                dst0 = a3[:, r * P3:r * P3 + 1]
                if r % 2 == 0 and (r // 2) % 3 != 2:
                    nc.scalar.activation(e3[:], ps3[:, 0:L3], ACTF.Relu,
                                         bias=cvcol(C_B3), scale=cvcol(C_S3))
                    nc.vector.tensor_max(dst1, e3[:, 1:380:2], e3[:, 2:381:2])
                    nc.vector.tensor_copy(dst0, e3[:, 0:1])
                else:
                    nc.vector.tensor_scalar(e3[:], ps3[:, 0:L3],
                                            cvcol(C_S3), cvcol(C_B3),
                                            op0=AL.mult, op1=AL.add)
                    nc.vector.scalar_tensor_tensor(
                        dst1, e3[:, 1:380:2], 0.0, e3[:, 2:381:2],
                        op0=AL.max, op1=AL.max)
                    nc.vector.tensor_scalar_max(dst0, e3[:, 0:1], 0.0)

            # prologue: t1 prefetch depth 3, conv1 runs 2 pairs ahead
            # pair 0's t1 split across two queues: halves move in parallel
            t10 = t1p.tile([6, 1502], F32R, tag="t1")
            t10v = t10.rearrange("k (pl c) -> k pl c", c=1502)
            nc.gpsimd.dma_start(
                t10v[:, :, 0:752],
                t1all_d[0:6, 0:752].rearrange("(pl k) c -> k pl c", k=6))
            nc.scalar.dma_start(
                t10v[:, :, 752:1502],
                t1all_d[0:6, 752:1502].rearrange("(pl k) c -> k pl c", k=6))
            t1s[0] = t10
            t1_load(1)
            t1_load(2)
            load_blob_tails()
            conv1(0)
            pa1(0)
            conv1(1)
            for p in range(PAIRS):
                # pool1+act1 for pair p+1 first: its conv1 ran last iteration
                if p + 1 < PAIRS:
                    pa1(p + 1)
                if p < 14:
                    wp_load(p)
                p1t = p1s.pop(p)
                ps2a = conv2(p1t, 0)
                s2a = pa2(ps2a, 0)
                ps2b = conv2(p1t, 1)
                s2b = pa2(ps2b, 1)
                if p + 3 < PAIRS:
                    t1_load(p + 3)
                if p + 2 < PAIRS:
                    conv1(p + 2)
                ps3a = conv3(s2a)
                pa3(ps3a, 2 * p)
                ps3b = conv3(s2b)
                pa3(ps3b, 2 * p + 1)

        # ======== map2 ========
        af = persist.tile([R, D2], F32)
        with (
            tc.tile_pool(name="mp2", bufs=1, space="PSUM") as mp2,
            tc.tile_pool(name="mtmp", bufs=1) as mtmp,
        ):
            psm = mp2.tile([R, D2], F32)
            for wi in range(N_WTILES):
                if wi not in wtiles:
                    wp_load(wi)
                wt = wtiles.pop(wi)
                l0 = W_TILE * wi
                nch = min(W_TILE, P3 - l0)
                for k in range(nch):
                    l = l0 + k
                    _mm(nc, psm[:], a3v[:, :, l], wt[:, D2 * k:D2 * (k + 1)],
                        start=(l == 0), stop=(l == P3 - 1))
            tmp = mtmp.tile([R, D2], F32)
            for h in range(2):
                cs = slice(128 * h, 128 * (h + 1))
                nc.vector.tensor_mul(tmp[:, cs], psm[:, cs],
                                     bfs("sm2rep", R)[:, cs])
                nc.vector.tensor_add(af[:, cs], tmp[:, cs],
                                     bfs("btot", R)[:, cs])

        # ======== MPNN blocks ========
        # prefetch fc1 weights up-front: SP/Pool stream them during MPNN
        f1ring = st.enter_context(tc.tile_pool(name="f1ring", bufs=1))
        wcs = []
        for i, row0 in enumerate([0, 512, 1024, 1536, 2048, 2560, 3072, 3584]):
            wc = f1ring.tile([128, 4 * D2], BF16, tag=f"f1w{i}")
            eng = nc.sync if i % 2 == 0 else nc.gpsimd
            eng.dma_start(
                wc.rearrange("p (n c) -> p n c", c=D2)[:],
                f1_d[row0:row0 + 512, :].rearrange("(n p) c -> p n c", p=128),
            )
            wcs.append(wc)

        o1 = persist.tile([128, NNODE * NG1], BF16)   # block out^T [c, (b,jw,n)]
        o2 = persist.tile([128, NNODE * NG2], BF16)
        with (
            tc.tile_pool(name="mp_sb", bufs=3) as msb,
            tc.tile_pool(name="mp_psf", bufs=2, space="PSUM") as mpsf,
            tc.tile_pool(name="mp_paw", bufs=2, space="PSUM") as mpaw,
            tc.tile_pool(name="mp_psy", bufs=1, space="PSUM") as mpsy,
            tc.tile_pool(name="mp_tp", bufs=1, space="PSUM") as mtp,
            tc.tile_pool(name="mp_tpa", bufs=1, space="PSUM") as mtpa,
            tc.tile_pool(name="mp_psh", bufs=1, space="PSUM") as mpsh,
        ):
            # AF^T chunks [128, 96] x2
            aft = []
            for h in range(2):
                pt = mtp.tile([128, R], F32, tag="tp")
                nc.tensor.transpose(pt[:], af[:, 128 * h:128 * (h + 1)],
                                    identsb[0:R, 0:R])
                t_ = msb.tile([128, R], F32R, tag=f"aft{h}")
                nc.vector.tensor_copy(t_[:], pt[:])
                aft.append(t_)

            for blk in range(2):
                graphs = GRAPHS1 if blk == 0 else GRAPHS2
                G = len(graphs)
                stride = 1 if blk == 0 else 2
                rbase = [NNODE * TLEN * b + NNODE * stride * j for (b, j) in graphs]
                gw = brs("g1w") if blk == 0 else brs("g2w")
                gbrep = bfs("g1brep", R) if blk == 0 else bfs("g2brep", R)
                thw = brs("th1w") if blk == 0 else brs("th2w")
                sa = (C_SA1_0, C_SA1_1) if blk == 0 else (C_SA2_0, C_SA2_1)
                ba = (C_BA1_0, C_BA1_1) if blk == 0 else (C_BA2_0, C_BA2_1)
                smc = C_SM1 if blk == 0 else C_SM2
                bmc = C_BM1 if blk == 0 else C_BM2
                ot = o1 if blk == 0 else o2

                # F = AF @ gw + gb   [96, 256]
                psf = mpsf.tile([R, D2], F32, tag="psf")
                for h in range(2):
                    _mm(nc, psf[:], aft[h][:], gw[:, D2 * h:D2 * (h + 1)],
                        start=(h == 0), stop=(h == 1))
                fsb = msb.tile([R, D2], F32, tag="fsb")
                nc.vector.tensor_add(fsb[:], psf[:], gbrep)

                # F^T chunks
                ft = []
                for h in range(2):
                    pt = mtp.tile([128, R], F32, tag="tp")
                    nc.tensor.transpose(pt[:], fsb[:, 128 * h:128 * (h + 1)],
                                        identsb[0:R, 0:R])
                    t_ = msb.tile([128, R], F32R, tag=f"ft{h}")
                    nc.vector.tensor_copy(t_[:], pt[:])
                    ft.append(t_)

                # per-graph Gram logits -> awps [8, 8G]
                awps = mpaw.tile([8, 8 * G], F32, tag="awps")
                for g, rb in enumerate(rbase):
                    for h in range(2):
                        _mm(nc, awps[:, 8 * g:8 * (g + 1)],
                            ft[h][:, rb:rb + 8], ft[h][:, rb:rb + 8],
                            start=(h == 0), stop=(h == 1))

                # Adj = softmax(lrelu(L - 1e8 eye)) * mask + eye   (rows = i)
                aw1 = msb.tile([8, 8 * G], F32, tag="aw1")
                nc.vector.tensor_add(aw1[:], awps[:], bfs("eyeneg", 8)[:, 0:8 * G])
                aw2 = msb.tile([8, 8 * G], F32, tag="aw2")
                nc.vector.scalar_tensor_tensor(aw2[:], aw1[:], 0.01, aw1[:],
                                               op0=AL.mult, op1=AL.max)
                a3d = aw2.rearrange("p (g j) -> p g j", j=8)
                rmax = msb.tile([8, G], F32, tag="rmax")
                nc.vector.reduce_max(rmax[:], a3d[:], axis=mybir.AxisListType.X)
                aw3 = msb.tile([8, 8 * G], F32, tag="aw3")
                nc.vector.tensor_sub(aw3.rearrange("p (g j) -> p g j", j=8)[:],
                                     a3d[:],
                                     rmax[:].unsqueeze(2).broadcast_to([8, G, 8]))
                aw4 = msb.tile([8, 8 * G], F32, tag="aw4")
                nc.scalar.activation(aw4[:], aw3[:], ACTF.Exp)
                rsum = msb.tile([8, G], F32, tag="rsum")
                nc.vector.reduce_sum(rsum[:],
                                     aw4.rearrange("p (g j) -> p g j", j=8)[:],
                                     axis=mybir.AxisListType.X)
                rrec = msb.tile([8, G], F32, tag="rrec")
                nc.vector.reciprocal(rrec[:], rsum[:])
                aw5 = msb.tile([8, 8 * G], F32, tag="aw5")
                nc.vector.tensor_mul(aw5.rearrange("p (g j) -> p g j", j=8)[:],
                                     aw4.rearrange("p (g j) -> p g j", j=8)[:],
                                     rrec[:].unsqueeze(2).broadcast_to([8, G, 8]))
                aw6 = msb.tile([8, 8 * G], F32, tag="aw6")
                nc.vector.tensor_mul(aw6[:], aw5[:], bfs("mask8", 8)[:, 0:8 * G])
                aw7 = msb.tile([8, 8 * G], F32, tag="aw7")
                nc.vector.tensor_add(aw7[:], aw6[:], bfs("eyepos", 8)[:, 0:8 * G])

                # Adj^T chunks via PE transpose -> adjtsb [<=128, 8]
                adjt = []
                for c0 in range(0, 8 * G, 128):
                    c1 = min(8 * G, c0 + 128)
                    pt = mtpa.tile([128, 8], F32, tag="tpa")
                    nc.tensor.transpose(pt[0:c1 - c0, :], aw7[:, c0:c1],
                                        identsb[0:8, 0:8])
                    t_ = msb.tile([128, 8], F32R, tag=f"adjt{c0}")
                    nc.vector.tensor_copy(t_[0:c1 - c0, :], pt[0:c1 - c0, :])
                    adjt.append(t_)

                # ADJBIG [96, 8G]: ADJBIG[rb_g + j, 8g + i] = Adj_g[i, j]
                adjbig = msb.tile([R, 8 * G], F32R, tag="adjbig")
                nc.vector.memset(adjbig[:].bitcast(F32), 0.0)
                for g, rb in enumerate(rbase):
                    src = adjt[g // 16][8 * (g % 16):8 * (g % 16) + 8, 0:8]
                    eng = (nc.sync, nc.gpsimd, nc.scalar)[g % 3]
                    eng.dma_start(adjbig[rb:rb + 8, 8 * g:8 * (g + 1)], src)

                # Xb^T = BN_A(AF^T); Y = Xb @ thw  [96, 128]
                xbt = []
                for h in range(2):
                    t_ = msb.tile([128, R], F32R, tag=f"xbt{h}")
                    nc.scalar.activation(t_[:], aft[h][:], ACTF.Identity,
                                         bias=cvcol(ba[h]), scale=cvcol(sa[h]))
                    xbt.append(t_)
                psy = mpsy.tile([R, HID], F32, tag="psy")
                for h in range(2):
                    _mm(nc, psy[:], xbt[h][:], thw[:, HID * h:HID * (h + 1)],
                        start=(h == 0), stop=(h == 1))
                ysb = msb.tile([R, HID], F32R, tag="ysb")
                nc.vector.tensor_copy(ysb[:], psy[:])

                # h^T: psh[c, 8g+i] = sum_r Y[r, c] ADJBIG[r, 8g+i]
                # per-batch column chunks: each starts right after that
                # batch's scatter DMAs instead of waiting for the full ADJBIG
                psh = mpsh.tile([128, 8 * G], F32, tag="psh")
                gpb = G // BSH          # graphs per batch (5 or 3)
                for b in range(BSH):
                    c0, c1 = 8 * gpb * b, 8 * gpb * (b + 1)
                    _mm(nc, psh[:, c0:c1], ysb[:], adjbig[:, c0:c1])

                # lrelu(psh * sM + bM)  (incl th_b, BN_M, x0.5 mean-fold)
                hpre = msb.tile([128, 8 * G], F32, tag="hpre")
                nc.scalar.activation(hpre[:], psh[:], ACTF.Identity,
                                     bias=cvcol(bmc), scale=cvcol(smc))
                hp = msb.tile([128, 8 * G], F32, tag="hp")
                nc.vector.scalar_tensor_tensor(hp[:], hpre[:], 0.01, hpre[:],
                                               op0=AL.mult, op1=AL.max)
                hpv = hp.rearrange("p (g j) -> p g j", j=8)
                nc.vector.tensor_add(ot.rearrange("p (g n) -> p g n", n=4)[:],
                                     hpv[:, :, 0:4], hpv[:, :, 4:8])

        # ======== FC head (fully transposed: no PSUM->SBUF copies,
        # no PE transposes; each layer h^T = W-chunk^T @ prev^T) ========
        with (
            tc.tile_pool(name="fc_sb", bufs=2) as fsb_p,
            tc.tile_pool(name="fc_ps", bufs=1, space="PSUM") as fps,
        ):
            o1v = o1.rearrange("p (b k) -> p b k", k=4 * 5)    # free = 20b + 4jw + n
            o2v = o2.rearrange("p (b k) -> p b k", k=4 * 3)

            def trelu(ps_list, bias_cols):
                outs = []
                for h, ps in enumerate(ps_list):
                    t_ = fsb_p.tile([128, BSH], F32R, tag=f"ht{bias_cols[h]}")
                    nc.scalar.activation(t_[:], ps[:], ACTF.Relu,
                                         bias=cvcol(bias_cols[h]), scale=1.0)
                    outs.append(t_)
                return outs

            h1ps = [fps.tile([128, BSH], F32, name=f"h1ps{h}",
                              tag=f"p1t{h}") for h in range(2)]
            ci = 0
            ti = 0
            for njw, o_v in ((5, o1v), (3, o2v)):
                for jw in range(njw):
                    wcv = wcs[ti].rearrange("p (n c) -> p n c", c=D2)
                    ti += 1
                    for n in range(NNODE):
                        for h in range(2):
                            _mm(nc, h1ps[h][:],
                                wcv[:, n, 128 * h:128 * (h + 1)],
                                o_v[:, :, 4 * jw + n],
                                start=(ci == 0), stop=(ci == 31))
                        ci += 1
            h1t = trelu(h1ps, (C_FB1_0, C_FB1_1))

            h2ps = [fps.tile([128, BSH], F32, name=f"h2ps{h}",
                              tag=f"p2t{h}") for h in range(2)]
            for mc in range(2):
                for kc in range(2):
                    _mm(nc, h2ps[mc][:],
                        brs("fc2w")[:, 256 * kc + 128 * mc:
                                    256 * kc + 128 * mc + 128],
                        h1t[kc][:], start=(kc == 0), stop=(kc == 1))
            h2t = trelu(h2ps, (C_FB2_0, C_FB2_1))

            h3ps = [fps.tile([128, BSH], F32, name="h3ps0", tag="p3t")]
            for kc in range(2):
                _mm(nc, h3ps[0][:], brs("fc3w")[:, 128 * kc:128 * (kc + 1)],
                    h2t[kc][:], start=(kc == 0), stop=(kc == 1))
            h3t = trelu(h3ps, (C_FB3,))
            ps4_ = fps.tile([BSH, 8], F32, tag="pfc4")
            _mm(nc, ps4_[:], h3t[0][:], brs("fc4w"))
            osb = fsb_p.tile([BSH, 8], F32, tag="osb")
            nc.vector.tensor_add(osb[:], ps4_[:], bfs("fb4", BSH))
            nc.sync.dma_start(out_d[:], osb[:, 0:5])

    nc.compile()
    return nc


_CACHE = {}


def _get_program():
    if "nc" not in _CACHE:
        _CACHE["nc"] = build_program()
    return _CACHE["nc"]


def make_in_maps(inputs):
    consts = prep_consts(inputs)
    x = np.asarray(inputs["X"], np.float32).reshape(BS * TLEN * NNODE, DIM)
    in_maps = []
    for c in range(NCORES):
        shard = x[R * c:R * (c + 1)]
        xp = np.zeros((R, 1504), np.float32)
        xp[:, 1:1 + DIM] = shard
        # t1all[3r + t, c] = xpad[r, c + t]
        sw = np.lib.stride_tricks.sliding_window_view(xp, 1502, axis=1)[:, 0:3]
        t1all = round_fp32r(sw.reshape(3 * R, 1502))
        m = {"t1all": t1all}
        m.update(consts)
        in_maps.append(m)
    return in_maps


def kernel(**inputs):
    from concourse.bass_utils import run_bass_kernel_spmd

    nc = _get_program()
    in_maps = make_in_maps(inputs)
    res = run_bass_kernel_spmd(nc, in_maps, core_ids=list(range(NCORES)))
    outs = [np.asarray(res.results[c]["out"]) for c in range(NCORES)]
    return np.concatenate(outs, axis=0).astype(np.float32)

